# revision 17
# baseline (speedup 1.0000x reference)
import sys

sys.path.insert(0, "/opt/trn_rl_repo")

import numpy as np

NCORES = 8
B, FULL_N, D = 4, 2048, 1024
NH = 16
DK = 64  # head dim
HPC = NH // NCORES  # heads per core = 2
CW = HPC * DK  # output columns per core = 128
DC = D // 128  # D chunks = 8
WSCALE = 16.0  # host-side weight scale so fp8 weights sit in normal range
PSCALE = 0.25  # exp output scale: keeps max prob value < fp8e4 max (240)

_CACHE = {}
LAST_RESULTS = None


def _build(n_rows):
    """SPMD Bass program for one core. Each core computes batch-0 attention
    for its 2 heads (the reference only uses att[0]) and adds it to its
    column slice of tgt for all batches.

    v6: fp8(e4m3) QKV projections and P.T@V in DoubleRow mode (2 contraction
    chunks per matmul). Exp is the scarce resource (ScalarE-only on trn2), so
    2 of 8 chunk-pairs per head go to DVE as a Schraudolph bits-exp (those
    chunk-pairs stay bf16 and use plain matmuls); ScalarE writes the rest as
    fp8 with a log(PSCALE) bias so e^6.7 fits e4m3. Both heads accumulate
    into separate PSUM banks in the same order exp output is produced, and
    next q-group's score matmuls are interleaved into the att stream so PE
    fills exp-wait gaps. Scores are bf16 with the two heads on disjoint PE
    row groups (auto tile_position) so their matmuls run concurrently."""
    import concourse.mybir as mybir
    import concourse.tile as tile
    from concourse import bacc
    from concourse.masks import make_identity

    fp32 = mybir.dt.float32
    bf16 = mybir.dt.bfloat16
    fp8 = mybir.dt.float8e4
    i16 = mybir.dt.int16
    DR = mybir.MatmulPerfMode.DoubleRow

    RT = n_rows // 128  # row tiles
    G = n_rows // 512  # 512-row groups
    QG = G
    KC = RT  # k chunks of 128
    JP = KC // 2  # chunk pairs per q-group = 8

    nc = bacc.Bacc(None, target_bir_lowering=False)
    tgt0t = nc.declare_dram_parameter("tgt0t", [D, n_rows], fp8, isOutput=False)
    mem0t = nc.declare_dram_parameter("mem0t", [D, n_rows], fp8, isOutput=False)
    wqt = nc.declare_dram_parameter("wqt", [D, CW], fp8, isOutput=False)
    wkt = nc.declare_dram_parameter("wkt", [D, CW], fp8, isOutput=False)
    wvt = nc.declare_dram_parameter("wvt", [D, CW], fp8, isOutput=False)
    tgtc = nc.declare_dram_parameter("tgtc", [B, n_rows, CW], bf16, isOutput=False)
    outc = nc.declare_dram_parameter("outc", [B, n_rows, CW], bf16, isOutput=True)

    Exp = mybir.ActivationFunctionType.Exp
    # Wq,Wk both carry x16 -> scores in PSUM are 256x; fold into exp scale.
    scale = 1.0 / (np.sqrt(DK) * WSCALE * WSCALE)
    pbias = float(np.log(PSCALE))
    # Schraudolph exp-as-bits (bf16 = top half of fp32), including PSCALE:
    # i16 = round(A*s + B); bits(i16) ~= PSCALE * exp(s*scale)
    SCH_A = float(128.0 * np.log2(np.e) * scale)
    SCH_B = float(128.0 * (127.0 - 0.0436775 + np.log2(PSCALE)))
    DVE_JP = (2, 4, 6)  # chunk pairs exp'd on DVE (bf16 P path)
    # DVE chunk-pair -> group holding its bf16 V chunks (always t=0,1)
    DVE_MAP = {2: 1, 4: 2, 6: 3}

    with tile.TileContext(nc) as tc:
        with (
            tc.tile_pool(name="const", bufs=1) as const,
            tc.tile_pool(name="persist", bufs=1) as persist,
        ):
            identb = const.tile([128, 128], bf16)
            make_identity(nc, identb)
            bias_sb = const.tile([128, 1], fp32, tag="pbias")
            nc.vector.memset(bias_sb, pbias)

            KT_gs = [
                persist.tile([128, 512], bf16, tag=f"KT{g}", name=f"KT{g}")
                for g in range(G)
            ]
            QT_gs = [
                persist.tile([128, 512], bf16, tag=f"QT{g}", name=f"QT{g}")
                for g in range(G)
            ]
            # fp8 V for DoubleRow P.T@V: [k%128, head, chunk-in-group, 80pad]
            # cols 0:64 = 16*v, col 64 = 16.0 (ones column -> row sums)
            Vp_gs = [
                persist.tile([128, HPC, 4, 80], fp8, tag=f"Vp{g}", name=f"Vp{g}")
                for g in range(G)
            ]
            # bf16 V copies only for the DVE-exp'd chunk pairs
            # (first two chunks of groups 1, 2, 3)
            Vb_gs = {
                g: persist.tile(
                    [128, HPC, 2, DK + 1], bf16, tag=f"Vb{g}", name=f"Vb{g}"
                )
                for g in (1, 2, 3)
            }
            att_sb = persist.tile([128, RT, CW], bf16, tag="att")
            tgtc_sb = persist.tile([128, B, RT, CW], bf16, tag="tgtc")

            with (
                tc.tile_pool(name="wst", bufs=1) as wst_pool,
                tc.tile_pool(name="grp", bufs=2) as grp_pool,
                tc.tile_pool(name="vtg", bufs=2) as vt_pool,
                tc.tile_pool(name="ps_acc", bufs=2, space="PSUM") as ps_acc,
            ):
                # PE warmup during the initial DMA wait. Real matmuls, not
                # transposes: transpose-mode does not count as PE-busy for
                # the HAM clock gate, so only genuine MMs un-throttle.
                pw = ps_acc.tile([128, 512], fp32, tag="acc")
                for i in range(30):
                    nc.tensor.matmul(
                        pw[:, 0:128], identb, identb,
                        start=(i == 0), stop=(i == 29),
                    )

                WTs = {}
                for name, w in (("q", wqt), ("k", wkt), ("v", wvt)):
                    wt = wst_pool.tile([128, DC, CW], fp8, tag=f"wt{name}")
                    # dispatched from the ACT queue: it is idle at startup,
                    # and this keeps the SP queue free for the memT halves
                    nc.scalar.dma_start(
                        out=wt, in_=w[:, :].rearrange("(c p) q -> p c q", p=128)
                    )
                    WTs[name] = wt

                def proj(dst, wname, src):
                    for d in range(DC // 2):
                        nc.tensor.matmul(
                            dst,
                            WTs[wname][:, 2 * d : 2 * d + 2, :],
                            src[:, 2 * d : 2 * d + 2, :],
                            start=(d == 0), stop=(d == DC // 2 - 1),
                            perf_mode=DR,
                        )

                def emit_mem_group(g):
                    memT_g = grp_pool.tile(
                        [128, DC, 512], fp8, tag="memTg", name=f"memT{g}"
                    )
                    for half in range(2):
                        nc.sync.dma_start(
                            out=memT_g[:, 4 * half : 4 * half + 4, :],
                            in_=mem0t[
                                512 * half : 512 * half + 512,
                                g * 512 : (g + 1) * 512,
                            ].rearrange("(c p) n -> p c n", p=128),
                        )
                    pk = ps_acc.tile([128, 512], fp32, tag="acc")
                    proj(pk, "k", memT_g)
                    nc.vector.tensor_copy(out=KT_gs[g], in_=pk)
                    pv = ps_acc.tile([128, 512], fp32, tag="acc")
                    proj(pv, "v", memT_g)
                    vt_g = vt_pool.tile([128, 512], bf16, tag="vtg")
                    nc.vector.tensor_copy(out=vt_g, in_=pv)
                    nc.vector.memset(Vp_gs[g], 16.0)
                    if g in Vb_gs:
                        nc.vector.memset(Vb_gs[g], 16.0)
                    for t in range(4):
                        ptr_t = ps_acc.tile([128, 512], fp32, tag="acc")
                        ptr = ptr_t[:, :].bitcast(bf16)
                        nc.tensor.transpose(
                            ptr[:, 0:128], vt_g[:, t * 128 : (t + 1) * 128], identb
                        )
                        for h in range(HPC):
                            nc.vector.tensor_copy(
                                out=Vp_gs[g][:, h, t, 0:DK],
                                in_=ptr[:, h * DK : (h + 1) * DK],
                            )
                        # bf16 copies for the chunks the DVE exp path uses
                        if g in Vb_gs and t < 2:
                            for h in range(HPC):
                                nc.vector.tensor_copy(
                                    out=Vb_gs[g][:, h, t, 0:DK],
                                    in_=ptr[:, h * DK : (h + 1) * DK],
                                )

                def emit_tgt_group(g):
                    tgtT_g = grp_pool.tile(
                        [128, DC, 512], fp8, tag="tgtTg", name=f"tgtT{g}"
                    )
                    for half in range(2):
                        nc.sync.dma_start(
                            out=tgtT_g[:, 4 * half : 4 * half + 4, :],
                            in_=tgt0t[
                                512 * half : 512 * half + 512,
                                g * 512 : (g + 1) * 512,
                            ].rearrange("(c p) n -> p c n", p=128),
                        )
                    pq = ps_acc.tile([128, 512], fp32, tag="acc")
                    proj(pq, "q", tgtT_g)
                    nc.vector.tensor_copy(out=QT_gs[g], in_=pq)

                with (
                    tc.tile_pool(name="pt", bufs=2) as pt_pool,
                    tc.tile_pool(name="usb", bufs=2) as usb_pool,
                    tc.tile_pool(name="small", bufs=8) as small_pool,
                    tc.tile_pool(name="ps_st", bufs=2, space="PSUM") as ps_st,
                    tc.tile_pool(name="ps_u", bufs=2, space="PSUM") as ps_u,
                ):
                    def alloc_pts(qg):
                        return [
                            {
                                "f8": pt_pool.tile(
                                    [128, KC, 512], fp8, tag=f"p8{h}",
                                    name=f"p8{h}_{qg}",
                                ),
                                "bf": pt_pool.tile(
                                    [128, 2 * len(DVE_JP), 512], bf16,
                                    tag=f"pb{h}", name=f"pb{h}_{qg}",
                                ),
                            }
                            for h in range(HPC)
                        ]

                    def emit_st_block(qg, pts, jp):
                        # scores for chunks 2jp, 2jp+1, both heads; heads on
                        # disjoint PE row groups -> concurrent matmuls
                        psts = [
                            ps_st.tile(
                                [128, 2, 512], fp32, tag="st",
                                name=f"st{qg}_{jp}_{h}",
                            )
                            for h in range(HPC)
                        ]
                        for jj in range(2):
                            j = jp * 2 + jj
                            kg, kt = j // 4, j % 4
                            for h in range(HPC):
                                hs = h * DK
                                nc.tensor.matmul(
                                    psts[h][:, jj, :],
                                    KT_gs[kg][hs : hs + DK, kt * 128 : (kt + 1) * 128],
                                    QT_gs[qg][hs : hs + DK, :],
                                    start=True, stop=True,
                                )
                        for h in range(HPC):
                            if jp in DVE_JP:
                                di = DVE_JP.index(jp)
                                nc.vector.tensor_scalar(
                                    out=pts[h]["bf"][
                                        :, 2 * di : 2 * di + 2, :
                                    ].bitcast(i16),
                                    in0=psts[h],
                                    scalar1=SCH_A,
                                    scalar2=SCH_B,
                                    op0=mybir.AluOpType.mult,
                                    op1=mybir.AluOpType.add,
                                )
                            else:
                                nc.scalar.activation(
                                    out=pts[h]["f8"][:, jp * 2 : jp * 2 + 2, :],
                                    in_=psts[h],
                                    func=Exp,
                                    scale=float(scale),
                                    bias=bias_sb[:, :],
                                )

                    all_pts = {}
                    # ramp: stream qg0's score chunks between memory groups so
                    # exp work starts as early as possible
                    emit_mem_group(0)
                    emit_tgt_group(0)
                    all_pts[0] = alloc_pts(0)
                    for jp in range(2):
                        emit_st_block(0, all_pts[0], jp)
                    for g in range(1, G):
                        emit_mem_group(g)
                        for jp in range(2 * g, 2 * g + 2):
                            emit_st_block(0, all_pts[0], jp)

                    for b in range(B):
                        nc.sync.dma_start(
                            out=tgtc_sb[:, b, :, :],
                            in_=tgtc[b, :, :].rearrange("(t p) c -> p t c", p=128),
                        )

                    for qg in range(QG):
                        if qg + 1 < QG:
                            emit_tgt_group(qg + 1)
                            all_pts[qg + 1] = alloc_pts(qg + 1)
                        qsl = slice(qg * 512, (qg + 1) * 512)
                        pts = all_pts[qg]
                        pus = [
                            ps_u.tile(
                                [DK + 1, 512], fp32, tag="u", name=f"pu{qg}_{h}"
                            )
                            for h in range(HPC)
                        ]
                        # att: both heads advance together, matching the exp
                        # production order; next q-group's scores interleave
                        for jp in range(JP):
                            for h in range(HPC):
                                if jp in DVE_JP:
                                    di = DVE_JP.index(jp)
                                    g = DVE_MAP[jp]
                                    for jj in range(2):
                                        nc.tensor.matmul(
                                            pus[h],
                                            Vb_gs[g][:, h, jj, :],
                                            pts[h]["bf"][:, 2 * di + jj, :],
                                            start=False,
                                            stop=False,
                                        )
                                else:
                                    g, jj = jp // 2, jp % 2
                                    nc.tensor.matmul(
                                        pus[h],
                                        Vp_gs[g][
                                            :, h, 2 * jj : 2 * jj + 2, 0 : DK + 1
                                        ],
                                        pts[h]["f8"][:, 2 * jp : 2 * jp + 2, :],
                                        start=(jp == 0),
                                        stop=(jp == JP - 1),
                                        perf_mode=DR,
                                    )
                            if qg + 1 < QG:
                                emit_st_block(qg + 1, all_pts[qg + 1], jp)
                        tail = qg == QG - 1
                        for h in range(HPC):
                            hs = h * DK
                            pu_sb = usb_pool.tile([DK + 1, 512], bf16, tag="usb")
                            if tail and h == 1:
                                # last q-group: ScalarE is idle; let it drain
                                # head 1 in parallel with DVE on head 0
                                nc.scalar.copy(out=pu_sb, in_=pus[h])
                            else:
                                nc.vector.tensor_copy(out=pu_sb, in_=pus[h])
                            for s in range(4):
                                pat_t = ps_acc.tile([128, 512], fp32, tag="acc")
                                pat = pat_t[:, :].bitcast(bf16)
                                nc.tensor.transpose(
                                    pat[:, 0 : DK + 1],
                                    pu_sb[:, s * 128 : (s + 1) * 128],
                                    identb[0 : DK + 1, 0 : DK + 1],
                                )
                                rec = small_pool.tile([128, 1], fp32, tag="rec")
                                nc.vector.reciprocal(rec, pat[:, DK : DK + 1])
                                if tail and h == 1:
                                    nc.scalar.mul(
                                        att_sb[:, qg * 4 + s, hs : hs + DK],
                                        pat[:, 0:DK],
                                        rec[:, :],
                                    )
                                else:
                                    nc.vector.tensor_scalar_mul(
                                        att_sb[:, qg * 4 + s, hs : hs + DK],
                                        in0=pat[:, 0:DK],
                                        scalar1=rec,
                                    )
                        # final broadcast add + store for this q-group's rows;
                        # adds go to gpsimd except the last q-group (tail)
                        add_eng = nc.vector if qg == QG - 1 else nc.gpsimd
                        for b in range(B):
                            add_eng.tensor_add(
                                out=tgtc_sb[:, b, qg * 4 : (qg + 1) * 4, :],
                                in0=tgtc_sb[:, b, qg * 4 : (qg + 1) * 4, :],
                                in1=att_sb[:, qg * 4 : (qg + 1) * 4, :],
                            )
                            nc.sync.dma_start(
                                out=outc[b, qsl, :].rearrange(
                                    "(t p) c -> p t c", p=128
                                ),
                                in_=tgtc_sb[:, b, qg * 4 : (qg + 1) * 4, :],
                            )

    nc.finalize()
    return nc


def _get_nc(n_rows):
    if n_rows not in _CACHE:
        _CACHE[n_rows] = _build(n_rows)
    return _CACHE[n_rows]


def _to_fp8(x):
    import ml_dtypes

    return np.ascontiguousarray(x, dtype=np.float32).astype(ml_dtypes.float8_e4m3)


def _to_bf16(x):
    import ml_dtypes

    return np.ascontiguousarray(x, dtype=np.float32).astype(ml_dtypes.bfloat16)


def _run(tgt, memory, Wq, Wk, Wv, trace=False):
    global LAST_RESULTS
    from concourse.bass_utils import run_bass_kernel_spmd

    n_rows = tgt.shape[1]
    nc = _get_nc(n_rows)

    tgt = np.ascontiguousarray(tgt, dtype=np.float32)
    memory = np.ascontiguousarray(memory, dtype=np.float32)
    tgt0t = _to_fp8(tgt[0].T)
    mem0t = _to_fp8(memory[0].T)

    in_maps = []
    for c in range(NCORES):
        sl = slice(c * CW, (c + 1) * CW)
        in_maps.append(
            {
                "tgt0t": tgt0t,
                "mem0t": mem0t,
                "wqt": _to_fp8(Wq[sl, :].T * WSCALE),
                "wkt": _to_fp8(Wk[sl, :].T * WSCALE),
                "wvt": _to_fp8(Wv[sl, :].T * WSCALE),
                "tgtc": _to_bf16(tgt[:, :, sl]),
            }
        )
    res = run_bass_kernel_spmd(nc, in_maps, list(range(NCORES)), trace=trace)
    LAST_RESULTS = res
    out = np.concatenate(
        [res.results[c]["outc"].astype(np.float32) for c in range(NCORES)], axis=2
    )
    return out


def kernel(tgt, memory, Wq, Wk, Wv):
    return _run(tgt, memory, Wq, Wk, Wv)


# revision 19
# speedup vs baseline: 1.0289x; 1.0289x over previous
import sys

sys.path.insert(0, "/opt/trn_rl_repo")

import numpy as np

NCORES = 8
B, FULL_N, D = 4, 2048, 1024
NH = 16
DK = 64  # head dim
HPC = NH // NCORES  # heads per core = 2
CW = HPC * DK  # output columns per core = 128
DC = D // 128  # D chunks = 8
WSCALE = 16.0  # host-side weight scale so fp8 weights sit in normal range
PSCALE = 0.25  # exp output scale: keeps max prob value < fp8e4 max (240)

_CACHE = {}
LAST_RESULTS = None


def _build(n_rows):
    """SPMD Bass program for one core. Each core computes batch-0 attention
    for its 2 heads (the reference only uses att[0]) and adds it to its
    column slice of tgt for all batches.

    v6: fp8(e4m3) QKV projections and P.T@V in DoubleRow mode (2 contraction
    chunks per matmul). Exp is the scarce resource (ScalarE-only on trn2), so
    2 of 8 chunk-pairs per head go to DVE as a Schraudolph bits-exp (those
    chunk-pairs stay bf16 and use plain matmuls); ScalarE writes the rest as
    fp8 with a log(PSCALE) bias so e^6.7 fits e4m3. Both heads accumulate
    into separate PSUM banks in the same order exp output is produced, and
    next q-group's score matmuls are interleaved into the att stream so PE
    fills exp-wait gaps. Scores are bf16 with the two heads on disjoint PE
    row groups (auto tile_position) so their matmuls run concurrently."""
    import concourse.mybir as mybir
    import concourse.tile as tile
    from concourse import bacc
    from concourse.masks import make_identity

    fp32 = mybir.dt.float32
    bf16 = mybir.dt.bfloat16
    fp8 = mybir.dt.float8e4
    i16 = mybir.dt.int16
    DR = mybir.MatmulPerfMode.DoubleRow

    RT = n_rows // 128  # row tiles
    G = n_rows // 512  # 512-row groups
    QG = G
    KC = RT  # k chunks of 128
    JP = KC // 2  # chunk pairs per q-group = 8

    nc = bacc.Bacc(None, target_bir_lowering=False)
    tgt0t = nc.declare_dram_parameter("tgt0t", [D, n_rows], fp8, isOutput=False)
    mem0t = nc.declare_dram_parameter("mem0t", [D, n_rows], fp8, isOutput=False)
    wqt = nc.declare_dram_parameter("wqt", [D, CW], fp8, isOutput=False)
    wkt = nc.declare_dram_parameter("wkt", [D, CW], fp8, isOutput=False)
    wvt = nc.declare_dram_parameter("wvt", [D, CW], fp8, isOutput=False)
    tgtc = nc.declare_dram_parameter("tgtc", [B, n_rows, CW], bf16, isOutput=False)
    outc = nc.declare_dram_parameter("outc", [B, n_rows, CW], bf16, isOutput=True)

    Exp = mybir.ActivationFunctionType.Exp
    # Wq,Wk both carry x16 -> scores in PSUM are 256x; fold into exp scale.
    scale = 1.0 / (np.sqrt(DK) * WSCALE * WSCALE)
    pbias = float(np.log(PSCALE))
    # Schraudolph exp-as-bits (bf16 = top half of fp32), including PSCALE:
    # i16 = round(A*s + B); bits(i16) ~= PSCALE * exp(s*scale)
    SCH_A = float(128.0 * np.log2(np.e) * scale)
    SCH_B = float(128.0 * (127.0 - 0.0436775 + np.log2(PSCALE)))
    DVE_JP = (2, 4, 6)  # chunk pairs exp'd on DVE (bf16 P path)
    # DVE chunk-pair -> group holding its bf16 V chunks (always t=0,1)
    DVE_MAP = {2: 1, 4: 2, 6: 3}

    with tile.TileContext(nc) as tc:
        with (
            tc.tile_pool(name="const", bufs=1) as const,
            tc.tile_pool(name="persist", bufs=1) as persist,
        ):
            identb = const.tile([128, 128], bf16)
            make_identity(nc, identb)
            bias_sb = const.tile([128, 1], fp32, tag="pbias")
            nc.vector.memset(bias_sb, pbias)

            KT_gs = [
                persist.tile([128, 512], bf16, tag=f"KT{g}", name=f"KT{g}")
                for g in range(G)
            ]
            QT_gs = [
                persist.tile([128, 512], bf16, tag=f"QT{g}", name=f"QT{g}")
                for g in range(G)
            ]
            # fp8 V for DoubleRow P.T@V: [k%128, head, chunk-in-group, 80pad]
            # cols 0:64 = 16*v, col 64 = 16.0 (ones column -> row sums)
            Vp_gs = [
                persist.tile([128, HPC, 4, 80], fp8, tag=f"Vp{g}", name=f"Vp{g}")
                for g in range(G)
            ]
            # bf16 V copies only for the DVE-exp'd chunk pairs
            # (first two chunks of groups 1, 2, 3)
            Vb_gs = {
                g: persist.tile(
                    [128, HPC, 2, DK + 1], bf16, tag=f"Vb{g}", name=f"Vb{g}"
                )
                for g in (1, 2, 3)
            }
            att_sb = persist.tile([128, RT, CW], bf16, tag="att")
            tgtc_sb = persist.tile([128, B, RT, CW], bf16, tag="tgtc")

            with (
                tc.tile_pool(name="wst", bufs=1) as wst_pool,
                tc.tile_pool(name="grp", bufs=4) as grp_pool,
                tc.tile_pool(name="vtg", bufs=2) as vt_pool,
                tc.tile_pool(name="ps_w", bufs=1, space="PSUM") as ps_w,
                tc.tile_pool(name="ps_acc", bufs=1, space="PSUM") as ps_acc,
            ):
                WTs = {}
                for name, w in (("q", wqt), ("k", wkt), ("v", wvt)):
                    wt = wst_pool.tile([128, DC, CW], fp8, tag=f"wt{name}")
                    # dispatched from the ACT queue: it is idle at startup,
                    # and this keeps the SP queue free for the memT halves
                    nc.scalar.dma_start(
                        out=wt, in_=w[:, :].rearrange("(c p) q -> p c q", p=128)
                    )
                    WTs[name] = wt

                def proj(dst, wname, src):
                    for d in range(DC // 2):
                        nc.tensor.matmul(
                            dst,
                            WTs[wname][:, 2 * d : 2 * d + 2, :],
                            src[:, 2 * d : 2 * d + 2, :],
                            start=(d == 0), stop=(d == DC // 2 - 1),
                            perf_mode=DR,
                        )

                memTs = {}

                def emit_k_group(g):
                    memT_g = grp_pool.tile(
                        [128, DC, 512], fp8, tag="memTg", name=f"memT{g}"
                    )
                    memTs[g] = memT_g
                    for half in range(2):
                        nc.sync.dma_start(
                            out=memT_g[:, 4 * half : 4 * half + 4, :],
                            in_=mem0t[
                                512 * half : 512 * half + 512,
                                g * 512 : (g + 1) * 512,
                            ].rearrange("(c p) n -> p c n", p=128),
                        )
                    pk = ps_acc.tile([128, 512], fp32, tag="acc")
                    proj(pk, "k", memT_g)
                    nc.vector.tensor_copy(out=KT_gs[g], in_=pk)

                def emit_v_group(g):
                    memT_g = memTs[g]
                    pv = ps_acc.tile([128, 512], fp32, tag="acc")
                    proj(pv, "v", memT_g)
                    vt_g = vt_pool.tile([128, 512], bf16, tag="vtg")
                    nc.vector.tensor_copy(out=vt_g, in_=pv)
                    nc.vector.memset(Vp_gs[g], 16.0)
                    if g in Vb_gs:
                        nc.vector.memset(Vb_gs[g], 16.0)
                    for t in range(4):
                        ptr = ps_w.tile([128, 128], bf16, tag="warm")
                        nc.tensor.transpose(
                            ptr, vt_g[:, t * 128 : (t + 1) * 128], identb
                        )
                        for h in range(HPC):
                            nc.vector.tensor_copy(
                                out=Vp_gs[g][:, h, t, 0:DK],
                                in_=ptr[:, h * DK : (h + 1) * DK],
                            )
                        # bf16 copies for the chunks the DVE exp path uses
                        if g in Vb_gs and t < 2:
                            for h in range(HPC):
                                nc.vector.tensor_copy(
                                    out=Vb_gs[g][:, h, t, 0:DK],
                                    in_=ptr[:, h * DK : (h + 1) * DK],
                                )

                def emit_tgt_group(g):
                    tgtT_g = grp_pool.tile(
                        [128, DC, 512], fp8, tag="tgtTg", name=f"tgtT{g}"
                    )
                    for half in range(2):
                        nc.sync.dma_start(
                            out=tgtT_g[:, 4 * half : 4 * half + 4, :],
                            in_=tgt0t[
                                512 * half : 512 * half + 512,
                                g * 512 : (g + 1) * 512,
                            ].rearrange("(c p) n -> p c n", p=128),
                        )
                    pq = ps_acc.tile([128, 512], fp32, tag="acc")
                    proj(pq, "q", tgtT_g)
                    nc.vector.tensor_copy(out=QT_gs[g], in_=pq)

                with (
                    tc.tile_pool(name="pt", bufs=2) as pt_pool,
                    tc.tile_pool(name="usb", bufs=2) as usb_pool,
                    tc.tile_pool(name="small", bufs=8) as small_pool,
                    tc.tile_pool(name="ps_st", bufs=2, space="PSUM") as ps_st,
                    tc.tile_pool(name="ps_u", bufs=2, space="PSUM") as ps_u,
                ):
                    def alloc_pts(qg):
                        return [
                            {
                                "f8": pt_pool.tile(
                                    [128, KC, 512], fp8, tag=f"p8{h}",
                                    name=f"p8{h}_{qg}",
                                ),
                                "bf": pt_pool.tile(
                                    [128, 2 * len(DVE_JP), 512], bf16,
                                    tag=f"pb{h}", name=f"pb{h}_{qg}",
                                ),
                            }
                            for h in range(HPC)
                        ]

                    def emit_st_block(qg, pts, jp):
                        # scores for chunks 2jp, 2jp+1, both heads; heads on
                        # disjoint PE row groups -> concurrent matmuls
                        psts = [
                            ps_st.tile(
                                [128, 2, 512], fp32, tag="st",
                                name=f"st{qg}_{jp}_{h}",
                            )
                            for h in range(HPC)
                        ]
                        for jj in range(2):
                            j = jp * 2 + jj
                            kg, kt = j // 4, j % 4
                            for h in range(HPC):
                                hs = h * DK
                                nc.tensor.matmul(
                                    psts[h][:, jj, :],
                                    KT_gs[kg][hs : hs + DK, kt * 128 : (kt + 1) * 128],
                                    QT_gs[qg][hs : hs + DK, :],
                                    start=True, stop=True,
                                )
                        for h in range(HPC):
                            if jp in DVE_JP:
                                di = DVE_JP.index(jp)
                                nc.vector.tensor_scalar(
                                    out=pts[h]["bf"][
                                        :, 2 * di : 2 * di + 2, :
                                    ].bitcast(i16),
                                    in0=psts[h],
                                    scalar1=SCH_A,
                                    scalar2=SCH_B,
                                    op0=mybir.AluOpType.mult,
                                    op1=mybir.AluOpType.add,
                                )
                            else:
                                nc.scalar.activation(
                                    out=pts[h]["f8"][:, jp * 2 : jp * 2 + 2, :],
                                    in_=psts[h],
                                    func=Exp,
                                    scale=float(scale),
                                    bias=bias_sb[:, :],
                                )

                    all_pts = {}
                    # ramp: K projections + qg0 scores first (exp is the
                    # critical resource - start it ASAP); V projections after,
                    # filling PE gaps while qg0's exp stream drains
                    emit_k_group(0)
                    emit_tgt_group(0)
                    all_pts[0] = alloc_pts(0)
                    for jp in range(2):
                        emit_st_block(0, all_pts[0], jp)
                    for g in range(1, G):
                        emit_k_group(g)
                        for jp in range(2 * g, 2 * g + 2):
                            emit_st_block(0, all_pts[0], jp)
                    for g in range(G):
                        emit_v_group(g)

                    for b in range(B):
                        nc.sync.dma_start(
                            out=tgtc_sb[:, b, :, :],
                            in_=tgtc[b, :, :].rearrange("(t p) c -> p t c", p=128),
                        )

                    for qg in range(QG):
                        if qg + 1 < QG:
                            emit_tgt_group(qg + 1)
                            all_pts[qg + 1] = alloc_pts(qg + 1)
                        qsl = slice(qg * 512, (qg + 1) * 512)
                        pts = all_pts[qg]
                        pus = [
                            ps_u.tile(
                                [DK + 1, 512], fp32, tag="u", name=f"pu{qg}_{h}"
                            )
                            for h in range(HPC)
                        ]
                        # att: both heads advance together, matching the exp
                        # production order; next q-group's scores interleave
                        for jp in range(JP):
                            for h in range(HPC):
                                if jp in DVE_JP:
                                    di = DVE_JP.index(jp)
                                    g = DVE_MAP[jp]
                                    for jj in range(2):
                                        nc.tensor.matmul(
                                            pus[h],
                                            Vb_gs[g][:, h, jj, :],
                                            pts[h]["bf"][:, 2 * di + jj, :],
                                            start=False,
                                            stop=False,
                                        )
                                else:
                                    g, jj = jp // 2, jp % 2
                                    nc.tensor.matmul(
                                        pus[h],
                                        Vp_gs[g][
                                            :, h, 2 * jj : 2 * jj + 2, 0 : DK + 1
                                        ],
                                        pts[h]["f8"][:, 2 * jp : 2 * jp + 2, :],
                                        start=(jp == 0),
                                        stop=(jp == JP - 1),
                                        perf_mode=DR,
                                    )
                            if qg + 1 < QG:
                                emit_st_block(qg + 1, all_pts[qg + 1], jp)
                        tail = qg == QG - 1
                        for h in range(HPC):
                            hs = h * DK
                            pu_sb = usb_pool.tile([DK + 1, 512], bf16, tag="usb")
                            if tail and h == 1:
                                # last q-group: ScalarE is idle; let it drain
                                # head 1 in parallel with DVE on head 0
                                nc.scalar.copy(out=pu_sb, in_=pus[h])
                            else:
                                nc.vector.tensor_copy(out=pu_sb, in_=pus[h])
                            for s in range(4):
                                pat = ps_w.tile([128, 128], bf16, tag="warm")
                                nc.tensor.transpose(
                                    pat[:, 0 : DK + 1],
                                    pu_sb[:, s * 128 : (s + 1) * 128],
                                    identb[0 : DK + 1, 0 : DK + 1],
                                )
                                rec = small_pool.tile([128, 1], fp32, tag="rec")
                                nc.vector.reciprocal(rec, pat[:, DK : DK + 1])
                                if tail and h == 1:
                                    nc.scalar.mul(
                                        att_sb[:, qg * 4 + s, hs : hs + DK],
                                        pat[:, 0:DK],
                                        rec[:, :],
                                    )
                                else:
                                    nc.vector.tensor_scalar_mul(
                                        att_sb[:, qg * 4 + s, hs : hs + DK],
                                        in0=pat[:, 0:DK],
                                        scalar1=rec,
                                    )
                        # final broadcast add + store for this q-group's rows;
                        # adds go to gpsimd except the last q-group (tail)
                        add_eng = nc.vector if qg == QG - 1 else nc.gpsimd
                        for b in range(B):
                            add_eng.tensor_add(
                                out=tgtc_sb[:, b, qg * 4 : (qg + 1) * 4, :],
                                in0=tgtc_sb[:, b, qg * 4 : (qg + 1) * 4, :],
                                in1=att_sb[:, qg * 4 : (qg + 1) * 4, :],
                            )
                            nc.sync.dma_start(
                                out=outc[b, qsl, :].rearrange(
                                    "(t p) c -> p t c", p=128
                                ),
                                in_=tgtc_sb[:, b, qg * 4 : (qg + 1) * 4, :],
                            )

    nc.finalize()
    return nc


def _get_nc(n_rows):
    if n_rows not in _CACHE:
        _CACHE[n_rows] = _build(n_rows)
    return _CACHE[n_rows]


def _to_fp8(x):
    import ml_dtypes

    return np.ascontiguousarray(x, dtype=np.float32).astype(ml_dtypes.float8_e4m3)


def _to_bf16(x):
    import ml_dtypes

    return np.ascontiguousarray(x, dtype=np.float32).astype(ml_dtypes.bfloat16)


def _run(tgt, memory, Wq, Wk, Wv, trace=False):
    global LAST_RESULTS
    from concourse.bass_utils import run_bass_kernel_spmd

    n_rows = tgt.shape[1]
    nc = _get_nc(n_rows)

    tgt = np.ascontiguousarray(tgt, dtype=np.float32)
    memory = np.ascontiguousarray(memory, dtype=np.float32)
    tgt0t = _to_fp8(tgt[0].T)
    mem0t = _to_fp8(memory[0].T)

    in_maps = []
    for c in range(NCORES):
        sl = slice(c * CW, (c + 1) * CW)
        in_maps.append(
            {
                "tgt0t": tgt0t,
                "mem0t": mem0t,
                "wqt": _to_fp8(Wq[sl, :].T * WSCALE),
                "wkt": _to_fp8(Wk[sl, :].T * WSCALE),
                "wvt": _to_fp8(Wv[sl, :].T * WSCALE),
                "tgtc": _to_bf16(tgt[:, :, sl]),
            }
        )
    res = run_bass_kernel_spmd(nc, in_maps, list(range(NCORES)), trace=trace)
    LAST_RESULTS = res
    out = np.concatenate(
        [res.results[c]["outc"].astype(np.float32) for c in range(NCORES)], axis=2
    )
    return out


def kernel(tgt, memory, Wq, Wk, Wv):
    return _run(tgt, memory, Wq, Wk, Wv)
